# revision 1
# baseline (speedup 1.0000x reference)
"""BERT(2-layer) + CRF NLL loss kernel for Trainium2, data-parallel over batch on 8 cores.

Layout strategy per core (2 examples, 1024 token-slots):
  - Activations kept feature-major in SBUF: hT [D=6x128 partitions, 1024 tokens].
    Linear layers then need no transposes: out_featmajor = lhsT(W).T @ hT,
    out_tokmajor = lhsT(hT_tile).T @ W.
  - LayerNorm over features = partition-axis reduction -> ones-matmuls on PE,
    mean/rstd broadcast back across partitions with ones-outer-product matmuls.
  - Attention: scoresT[k,q] per (example,head) with k on partitions; exp without
    max-subtraction (scores are tiny: |s|<~2); denominator = extra ones-row in the
    ctx matmul; normalization folded in with a reciprocal + broadcast multiply.
  - CRF forward scan in log domain as an associative product of 9x9 matrices:
    M_t[i,j] = trans[i,j] + e_t[j] (identity_log where masked). 510 steps are
    grouped into 64 chunks x 8 steps per example (128 chunk-partitions total),
    combined sequentially within chunks and by a binary tree across partitions.
  - Matmuls in bf16 (validated on host: final-loss rel err ~2e-5); LN / softmax
    normalization / CRF in fp32.
"""

import sys

sys.path.insert(0, "/opt/trn_rl_repo")

import numpy as np
import ml_dtypes

import concourse.bass as bass
import concourse.tile as tile
from concourse import bacc, mybir
from concourse.bass import AP
from concourse.bass_utils import run_bass_kernel_spmd
from concourse.masks import make_identity

F32 = mybir.dt.float32
BF16 = mybir.dt.bfloat16
I32 = mybir.dt.int32
AF = mybir.ActivationFunctionType
ALU = mybir.AluOpType
AX = mybir.AxisListType

P = 128
B, S, D, L, H, T, V = 16, 512, 768, 2, 12, 9, 30522
DH = D // H          # 64
FF = 4 * D           # 3072
NCORES = 8
BL = B // NCORES     # 2 examples per core
NTOK = BL * S        # 1024
KD = D // P          # 6 k-tiles over D
KF = FF // P         # 24 k-tiles over FF
NT = NTOK // 512     # 2 n-chunks of 512 tokens
TT = NTOK // P       # 8 token-tiles
EPS = 1e-12
NEG = -1000.0        # effective -inf for log-domain CRF
G = 8                # CRF scan steps per chunk
CCH = 64             # chunks per example
NSTEP = 510          # scan steps (S'-1 where S'=511)
EMROWS = NTOK + 16   # em output padded so chunk loads never go OOB

def _bitrev(n, bits):
    r = 0
    for _ in range(bits):
        r = (r << 1) | (n & 1)
        n >>= 1
    return r

_BITREV7 = np.array([_bitrev(p, 7) for p in range(128)], dtype=np.int64)


# ----------------------------------------------------------------------------
# device program
# ----------------------------------------------------------------------------

def build_program():
    nc = bacc.Bacc("TRN2", target_bir_lowering=False, debug=False)

    def din(name, shape, dt):
        return nc.dram_tensor(name, shape, dt, kind="ExternalInput").ap()

    def dout(name, shape, dt):
        return nc.dram_tensor(name, shape, dt, kind="ExternalOutput").ap()

    t = dict(
        tok=din("tok", [NTOK, 1], I32),
        wemb=din("wemb", [V, D], F32),
        pos=din("pos", [S, D], F32),
        lnesB=din("lnesB", [P, D], F32),
        lnebB=din("lnebB", [P, D], F32),
        wqkv=din("wqkv", [L, D, 3 * D], BF16),
        wo=din("wo", [L, D, D], BF16),
        w1=din("w1", [L, D, FF], BF16),
        w2=din("w2", [L, FF, D], BF16),
        wtag=din("wtag", [D, T], BF16),
        bqkvT=din("bqkvT", [L, P, 18], F32),
        bvB=din("bvB", [L, P, D], F32),
        boT=din("boT", [L, P, KD], F32),
        b1T=din("b1T", [L, P, KF], F32),
        b2T=din("b2T", [L, P, KD], F32),
        ln1sT=din("ln1sT", [L, P, KD], F32),
        ln1bT=din("ln1bT", [L, P, KD], F32),
        ln2sT=din("ln2sT", [L, P, KD], F32),
        ln2bT=din("ln2bT", [L, P, KD], F32),
        btag=din("btag", [T, 1], F32),
        transB=din("transB", [P, 81], F32),
        ilogB=din("ilogB", [P, 81], F32),
        maskB=din("maskB", [P, G], F32),
        start2=din("start2", [BL, T], F32),
        end2=din("end2", [BL, T], F32),
        selT=din("selT", [T, NTOK], F32),
        permC=din("permC", [P, 1], I32),
        emS=nc.dram_tensor("emS", [P, G * T], F32, kind="Internal").ap(),
        em=dout("em", [EMROWS, T], F32),
        numdot=dout("numdot", [T, 1], F32),
        logz=dout("logz", [BL, 1], F32),
    )

    with tile.TileContext(nc) as tc:
        _emit(nc, tc, t)
    nc.compile()
    return nc


def _emit(nc, tc, t):
    from contextlib import ExitStack

    with ExitStack() as ctx:
        const = ctx.enter_context(tc.tile_pool(name="const", bufs=1))
        hpool = ctx.enter_context(tc.tile_pool(name="h", bufs=1))

        ident = const.tile([P, P], F32, name="ident", tag="ident")
        make_identity(nc, ident[:])
        ones_bf = const.tile([P, 1], BF16, name="ones_bf", tag="ones_bf")
        nc.vector.memset(ones_bf[:], 1.0)
        ones1 = const.tile([1, P], F32, name="ones1", tag="ones1")      # bcast lhsT
        nc.vector.memset(ones1[:], 1.0)
        ones128 = const.tile([P, 1], F32, name="ones128", tag="ones128")  # LN-sum lhsT
        nc.vector.memset(ones128[:], 1.0)
        ones128b = const.tile([P, 1], BF16, name="ones128b", tag="ones128b")
        nc.vector.memset(ones128b[:], 1.0)
        epsc = const.tile([P, 1], F32, name="epsc", tag="epsc")
        nc.vector.memset(epsc[:], EPS)
        lnesB_s = const.tile([P, D], F32, name="lnesB", tag="lnesB")
        nc.sync.dma_start(lnesB_s[:], t["lnesB"][:])
        lnebB_s = const.tile([P, D], F32, name="lnebB", tag="lnebB")
        nc.sync.dma_start(lnebB_s[:], t["lnebB"][:])

        # persistent activation tiles
        hT = [hpool.tile([P, NTOK], F32, name=f"hT{d}", tag=f"hT{d}") for d in range(KD)]
        hbf = [hpool.tile([P, NTOK], BF16, name=f"hbf{d}", tag=f"hbf{d}") for d in range(KD)]
        qkT = [hpool.tile([P, NTOK], BF16, name=f"qkT{d}", tag=f"qkT{d}") for d in range(2 * KD)]
        vtm = [hpool.tile([P, H * (DH + 1)], BF16, name=f"vtm{m}", tag=f"vtm{m}")
               for m in range(TT)]
        for m in range(TT):
            ones_col = vtm[m][:].rearrange("p (h c) -> p h c", c=DH + 1)[:, :, DH:]
            nc.vector.memset(ones_col, 1.0)
        ctxT = [hpool.tile([P, NTOK], BF16, name=f"ctxT{d}", tag=f"ctxT{d}") for d in range(KD)]

        # ------------------------------------------------------------------
        # embedding: gather + pos + LN (token-major), then transpose to hT
        # ------------------------------------------------------------------
        with tc.tile_pool(name="emb", bufs=3) as emb, \
             tc.tile_pool(name="embps", bufs=3, space="PSUM") as embps, \
             tc.tile_pool(name="posp", bufs=1) as posp:
            pos_t = []
            for q in range(S // P):
                pt = posp.tile([P, D], F32, name=f"pos{q}", tag=f"pos{q}")
                nc.sync.dma_start(pt[:], t["pos"][q * P:(q + 1) * P, :])
                pos_t.append(pt)
            for tt_i in range(TT):
                idx = emb.tile([P, 1], I32, name="idx", tag="idx")
                nc.sync.dma_start(idx[:], t["tok"][tt_i * P:(tt_i + 1) * P, :])
                g = emb.tile([P, D], F32, name="g", tag="g")
                nc.gpsimd.indirect_dma_start(
                    out=g[:], out_offset=None, in_=t["wemb"][:],
                    in_offset=bass.IndirectOffsetOnAxis(ap=idx[:, :1], axis=0),
                )
                nc.vector.tensor_add(g[:], g[:], pos_t[tt_i % (S // P)][:])
                # token-major layernorm
                mu = emb.tile([P, 1], F32, name="mu", tag="mu")
                nc.vector.reduce_sum(out=mu[:], in_=g[:], axis=AX.X)
                nc.vector.tensor_scalar_mul(mu[:], mu[:], 1.0 / D)
                cen = emb.tile([P, D], F32, name="cen", tag="cen")
                nc.vector.tensor_scalar(
                    out=cen[:], in0=g[:], scalar1=mu[:, :1], scalar2=None,
                    op0=ALU.subtract)
                sq = emb.tile([P, D], F32, name="sq", tag="sq")
                nc.vector.tensor_mul(sq[:], cen[:], cen[:])
                var = emb.tile([P, 1], F32, name="var", tag="var")
                nc.vector.reduce_sum(out=var[:], in_=sq[:], axis=AX.X)
                nc.vector.tensor_scalar_mul(var[:], var[:], 1.0 / D)
                sd = emb.tile([P, 1], F32, name="sd", tag="sd")
                nc.scalar.activation(sd[:], var[:], AF.Sqrt, bias=epsc[:sd.shape[0], :1])
                rs = emb.tile([P, 1], F32, name="rs", tag="rs")
                nc.vector.reciprocal_approx_fast(rs[:], sd[:])
                nc.vector.tensor_scalar_mul(cen[:], cen[:], rs[:, :1])
                nc.vector.tensor_mul(cen[:], cen[:], lnesB_s[:])
                nc.vector.tensor_add(cen[:], cen[:], lnebB_s[:])
                # transpose [128tok, 768] -> hT[d][:, tt*128...]
                for d in range(KD):
                    tp = embps.tile([P, P], F32, name="tp", tag="tp", space="PSUM")
                    nc.tensor.transpose(tp[:], cen[:, d * P:(d + 1) * P], ident[:])
                    nc.vector.tensor_copy(
                        hT[d][:, tt_i * P:(tt_i + 1) * P], tp[:])
        for d in range(KD):
            nc.vector.tensor_copy(hbf[d][:], hT[d][:])

        # ------------------------------------------------------------------
        # encoder layers
        # ------------------------------------------------------------------
        with tc.tile_pool(name="wA", bufs=6) as wA, \
             tc.tile_pool(name="wB", bufs=8) as wB, \
             tc.tile_pool(name="wC", bufs=6) as wC:
            for l in range(L):
                _layer(nc, tc, t, l, hT, hbf, qkT, vtm, ctxT,
                       wA, wB, wC, ones_bf, ones1, ones128, ones128b, epsc)

        # ------------------------------------------------------------------
        # emissions: emT = wtag.T @ hbf + btag  (feature-major [9, NTOK])
        # ------------------------------------------------------------------
        with tc.tile_pool(name="emp", bufs=1) as emp, \
             tc.tile_pool(name="emps", bufs=2, space="PSUM") as emps:
            wtg = emp.tile([P, KD, T], BF16, name="wtg", tag="wtg")
            nc.sync.dma_start(
                wtg[:], t["wtag"].rearrange("(k p) t -> p k t", p=P))
            btg = emp.tile([T, 1], F32, name="btg", tag="btg")
            nc.sync.dma_start(btg[:], t["btag"][:])
            em_sb = emp.tile([T, NTOK], F32, name="em_sb", tag="em_sb")
            for n in range(NT):
                ps = emps.tile([T, 512], F32, name="emmm", tag="emmm", space="PSUM")
                for k in range(KD):
                    nc.tensor.matmul(
                        ps[:], lhsT=wtg[:, k, :],
                        rhs=hbf[k][:, n * 512:(n + 1) * 512],
                        start=(k == 0), stop=(k == KD - 1))
                nc.scalar.activation(
                    em_sb[:, n * 512:(n + 1) * 512], ps[:], AF.Identity,
                    bias=btg[:, :1], scale=1.0)
            # numerator dot: sum(em * selT)
            sel = emp.tile([T, NTOK], F32, name="sel", tag="sel")
            nc.sync.dma_start(sel[:], t["selT"][:])
            prod = emp.tile([T, NTOK], F32, name="prod", tag="prod")
            nc.vector.tensor_mul(prod[:], em_sb[:], sel[:])
            nd = emp.tile([T, 1], F32, name="nd", tag="nd")
            nc.vector.reduce_sum(out=nd[:], in_=prod[:], axis=AX.X)
            nc.sync.dma_start(t["numdot"][:], nd[:])
            # token-major em to DRAM (+ zero pad rows)
            zpad = emp.tile([16, T], F32, name="zpad", tag="zpad")
            nc.vector.memset(zpad[:], 0.0)
            nc.sync.dma_start(t["em"][NTOK:EMROWS, :], zpad[:])
            for tt_i in range(TT):
                tp = emps.tile([P, T], F32, name="emtp", tag="emtp", space="PSUM")
                nc.tensor.transpose(
                    tp[:], em_sb[:, tt_i * P:(tt_i + 1) * P], ident[:T, :T])
                emtm = emp.tile([P, T], F32, name="emtm", tag="emtm", bufs=3)
                nc.vector.tensor_copy(emtm[:], tp[:])
                nc.sync.dma_start(t["em"][tt_i * P:(tt_i + 1) * P, :], emtm[:])

        # ------------------------------------------------------------------
        # CRF forward pass (log-domain associative scan)
        # ------------------------------------------------------------------
        _crf(nc, tc, t)


def _ln_feature_major(nc, tc, hT, hbf, ones128, ones1, sT, bT, epsc):
    """In-place layernorm of hT over the feature (partition) axis; refresh hbf.

    sT/bT: [128, KD] per-partition scale/bias tiles.
    """
    with tc.tile_pool(name="lnp", bufs=1) as lnp, \
         tc.tile_pool(name="lnps", bufs=2, space="PSUM") as lnps:
        for n in range(NT):
            sl = slice(n * 512, (n + 1) * 512)
            for k in range(KD):
                nc.vector.tensor_copy(hbf[k][:, sl], hT[k][:, sl])
            mu_ps = lnps.tile([1, 512], F32, name="mu", tag="mu", space="PSUM")
            sq_ps = lnps.tile([1, 512], F32, name="sq", tag="sq", space="PSUM")
            xsq = [lnp.tile([P, 512], BF16, name=f"xsq{k}", tag=f"xsq{k}")
                   for k in range(KD)]
            for k in range(KD):
                nc.vector.tensor_mul(xsq[k][:], hbf[k][:, sl], hbf[k][:, sl])
            for k in range(KD):
                nc.tensor.matmul(mu_ps[:], lhsT=ones128[:], rhs=hbf[k][:, sl],
                                 start=(k == 0), stop=(k == KD - 1))
            for k in range(KD):
                nc.tensor.matmul(sq_ps[:], lhsT=ones128[:], rhs=xsq[k][:],
                                 start=(k == 0), stop=(k == KD - 1))
            mu = lnp.tile([1, 512], F32, name="mus", tag="mus", bufs=2)
            nc.vector.tensor_scalar_mul(mu[:], mu_ps[:], 1.0 / D)
            msq = lnp.tile([1, 512], F32, name="msqs", tag="msqs", bufs=2)
            nc.vector.tensor_scalar_mul(msq[:], sq_ps[:], 1.0 / D)
            var = lnp.tile([1, 512], F32, name="vars", tag="vars", bufs=2)
            nc.vector.tensor_mul(var[:], mu[:], mu[:])
            nc.vector.tensor_sub(var[:], msq[:], var[:])
            sd = lnp.tile([1, 512], F32, name="sds", tag="sds", bufs=2)
            nc.scalar.activation(sd[:], var[:], AF.Sqrt, bias=epsc[:1, :1])
            rs = lnp.tile([1, 512], F32, name="rss", tag="rss", bufs=2)
            nc.vector.reciprocal_approx_fast(rs[:], sd[:])
            muB = lnps.tile([P, 512], F32, name="muB", tag="muB", space="PSUM")
            nc.tensor.matmul(muB[:], lhsT=ones1[:], rhs=mu[:],
                             start=True, stop=True)
            rsB = lnps.tile([P, 512], F32, name="rsB", tag="rsB", space="PSUM")
            nc.tensor.matmul(rsB[:], lhsT=ones1[:], rhs=rs[:],
                             start=True, stop=True)
            for k in range(KD):
                tmp = lnp.tile([P, 512], F32, name="tmp", tag="tmp", bufs=3)
                nc.vector.tensor_sub(tmp[:], hT[k][:, sl], muB[:])
                nc.vector.tensor_mul(tmp[:], tmp[:], rsB[:])
                nc.scalar.activation(hT[k][:, sl], tmp[:], AF.Identity,
                                     bias=bT[:, k:k + 1], scale=sT[:, k:k + 1])
                nc.vector.tensor_copy(hbf[k][:, sl], hT[k][:, sl])


def _layer(nc, tc, t, l, hT, hbf, qkT, vtm, ctxT, wA, wB, wC,
           ones_bf, ones1, ones128, ones128b, epsc):
    # per-layer bias/param tiles
    with tc.tile_pool(name=f"par{l}", bufs=1) as par:
        bqkv_t = par.tile([P, 18], F32, name="bqkv", tag="bqkv")
        nc.sync.dma_start(bqkv_t[:], t["bqkvT"][l])
        bv_t = par.tile([P, D], F32, name="bv", tag="bv")
        nc.sync.dma_start(bv_t[:], t["bvB"][l])
        bo_t = par.tile([P, KD], F32, name="bo", tag="bo")
        nc.sync.dma_start(bo_t[:], t["boT"][l])
        b1_t = par.tile([P, KF], F32, name="b1", tag="b1")
        nc.sync.dma_start(b1_t[:], t["b1T"][l])
        b2_t = par.tile([P, KD], F32, name="b2", tag="b2")
        nc.sync.dma_start(b2_t[:], t["b2T"][l])
        ln1s_t = par.tile([P, KD], F32, name="ln1s", tag="ln1s")
        nc.sync.dma_start(ln1s_t[:], t["ln1sT"][l])
        ln1b_t = par.tile([P, KD], F32, name="ln1b", tag="ln1b")
        nc.sync.dma_start(ln1b_t[:], t["ln1bT"][l])
        ln2s_t = par.tile([P, KD], F32, name="ln2s", tag="ln2s")
        nc.sync.dma_start(ln2s_t[:], t["ln2sT"][l])
        ln2b_t = par.tile([P, KD], F32, name="ln2b", tag="ln2b")
        nc.sync.dma_start(ln2b_t[:], t["ln2bT"][l])

        # --------------- QK (feature-major) + V (token-major) --------------
        wq = []
        for k in range(KD):
            wt = wA.tile([P, 3 * D], BF16, name="wqkv", tag="wqkv")
            nc.sync.dma_start(wt[:], t["wqkv"][l, k * P:(k + 1) * P, :])
            wq.append(wt)
        with tc.tile_pool(name="qkps", bufs=3, space="PSUM") as qkps:
            for n in range(NT):
                for m in range(2 * KD):       # QK output feature tiles
                    ps = qkps.tile([P, 512], F32, name="ps", tag="ps", space="PSUM")
                    for k in range(KD):
                        nc.tensor.matmul(
                            ps[:], lhsT=wq[k][:, m * P:(m + 1) * P],
                            rhs=hbf[k][:, n * 512:(n + 1) * 512],
                            start=(k == 0), stop=(k == KD - 1))
                    nc.vector.tensor_scalar_add(
                        qkT[m][:, n * 512:(n + 1) * 512], ps[:],
                        bqkv_t[:, m:m + 1])
            for m in range(TT):               # V token-major tiles
                for n in range(2):
                    nsl = slice(2 * D + n * 384, 2 * D + (n + 1) * 384)
                    vsl = slice(n * 384, (n + 1) * 384)
                    ps = qkps.tile([P, 384], F32, name="psv", tag="psv", space="PSUM")
                    for k in range(KD):
                        nc.tensor.matmul(
                            ps[:], lhsT=hbf[k][:, m * P:(m + 1) * P],
                            rhs=wq[k][:, nsl],
                            start=(k == 0), stop=(k == KD - 1))
                    vdst = vtm[m][:].rearrange(
                        "p (h c) -> p h c", c=DH + 1)[:, n * 6:(n + 1) * 6, :DH]
                    nc.vector.tensor_add(
                        vdst, ps[:].rearrange("p (h c) -> p h c", c=DH),
                        bv_t[:, vsl].rearrange("p (h c) -> p h c", c=DH))

        # --------------- attention ----------------------------------------
        with tc.tile_pool(name="att", bufs=1) as att, \
             tc.tile_pool(name="attp", bufs=3, space="PSUM") as attp, \
             tc.tile_pool(name="ctxp", bufs=2, space="PSUM") as ctxp, \
             tc.tile_pool(name="invp", bufs=2, space="PSUM") as invp:
            for b in range(BL):
                bsl = slice(b * S, (b + 1) * S)
                for hp in range(H // 2):      # head pairs
                    cps = []
                    for hh in range(2):
                        h = hp * 2 + hh
                        dt_i = h // 2
                        po = (h % 2) * DH     # partition offset inside tile
                        qsl = slice(po, po + DH)
                        expt = []
                        for kt in range(4):
                            ps = attp.tile([P, S], F32, name="sc", tag="sc", space="PSUM")
                            ksl = slice(b * S + kt * P, b * S + (kt + 1) * P)
                            nc.tensor.matmul(
                                ps[:], lhsT=qkT[KD + dt_i][qsl, ksl],
                                rhs=qkT[dt_i][qsl, bsl],
                                start=True, stop=True)
                            et = att.tile([P, S], BF16, name="expt", tag="expt", bufs=8)
                            nc.scalar.activation(et[:], ps[:], AF.Exp,
                                                 scale=0.125)
                            expt.append(et)
                        cp = ctxp.tile([P, S], F32, name="ctx", tag="ctx", space="PSUM")
                        for kt in range(4):
                            vt = vtm[b * 4 + kt]
                            nc.tensor.matmul(
                                cp[:DH + 1, :],
                                lhsT=vt[:, h * (DH + 1):(h + 1) * (DH + 1)],
                                rhs=expt[kt][:], start=(kt == 0),
                                stop=(kt == 3))
                        cps.append(cp)
                    # normalize the pair into ctxT
                    ivB = invp.tile([P, S], F32, name="ivB", tag="ivB", space="PSUM")
                    iv_sb = []
                    for hh in range(2):
                        dnm = att.tile([1, S], F32, name="dnm", tag="dnm", bufs=4)
                        nc.vector.tensor_copy(dnm[:], cps[hh][DH:DH + 1, :])
                        iv = att.tile([1, S], F32, name="iv", tag="iv", bufs=4)
                        nc.vector.reciprocal_approx_fast(iv[:], dnm[:])
                        iv_sb.append(iv)
                    nc.tensor.matmul(ivB[:DH, :], lhsT=ones1[:, :DH],
                                     rhs=iv_sb[0][:], start=True, stop=True)
                    nc.tensor.matmul(ivB[DH:, :], lhsT=ones1[:, :DH],
                                     rhs=iv_sb[1][:], start=True, stop=True)
                    ivS = att.tile([P, S], F32, name="ivS", tag="ivS", bufs=2)
                    nc.scalar.copy(ivS[:], ivB[:])
                    for hh in range(2):
                        nc.vector.tensor_mul(
                            ctxT[hp][hh * DH:(hh + 1) * DH, bsl],
                            cps[hh][:DH, :], ivS[hh * DH:(hh + 1) * DH, :])

        # --------------- Wo + residual -------------------------------------
        wo_t = []
        for k in range(KD):
            wt = wB.tile([P, D], BF16, name="wB", tag="wB")
            nc.sync.dma_start(wt[:], t["wo"][l, k * P:(k + 1) * P, :])
            wo_t.append(wt)
        with tc.tile_pool(name="wop", bufs=3, space="PSUM") as wop, \
             tc.tile_pool(name="wos", bufs=3) as wos:
            for n in range(NT):
                for m in range(KD):
                    sl = slice(n * 512, (n + 1) * 512)
                    ps = wop.tile([P, 512], F32, name="ps", tag="ps", space="PSUM")
                    for k in range(KD):
                        nc.tensor.matmul(
                            ps[:], lhsT=wo_t[k][:, m * P:(m + 1) * P],
                            rhs=ctxT[k][:, sl],
                            start=(k == 0), stop=(k == KD - 1))
                    tmp = wos.tile([P, 512], F32, name="tmp", tag="tmp")
                    nc.vector.tensor_scalar_add(tmp[:], ps[:], bo_t[:, m:m + 1])
                    nc.vector.tensor_add(hT[m][:, sl], hT[m][:, sl], tmp[:])
        _ln_feature_major(nc, tc, hT, hbf, ones128b, ones1, ln1s_t, ln1b_t, epsc)

        # --------------- FF -------------------------------------------------
        w1_t = []
        for k in range(KD):
            wt = wC.tile([P, FF], BF16, name="wC", tag="wC")
            nc.sync.dma_start(wt[:], t["w1"][l, k * P:(k + 1) * P, :])
            w1_t.append(wt)
        with tc.tile_pool(name="ffg", bufs=8) as ffg, \
             tc.tile_pool(name="ffps", bufs=2, space="PSUM") as ffps, \
             tc.tile_pool(name="ffac", bufs=1, space="PSUM") as ffac, \
             tc.tile_pool(name="ffs", bufs=3) as ffs:
            for n in range(NT):
                sl = slice(n * 512, (n + 1) * 512)
                acc = [ffac.tile([P, 512], F32, name=f"acc{m}", tag=f"acc{m}", space="PSUM")
                       for m in range(KD)]
                for kk in range(KF):
                    w2t = wB.tile([P, D], BF16, name="wB", tag="wB")
                    nc.sync.dma_start(
                        w2t[:], t["w2"][l, kk * P:(kk + 1) * P, :])
                    psg = ffps.tile([P, 512], F32, name="psg", tag="psg", space="PSUM")
                    for k in range(KD):
                        nc.tensor.matmul(
                            psg[:], lhsT=w1_t[k][:, kk * P:(kk + 1) * P],
                            rhs=hbf[k][:, sl],
                            start=(k == 0), stop=(k == KD - 1))
                    gl = ffg.tile([P, 512], BF16, name="gl", tag="gl")
                    nc.scalar.activation(gl[:], psg[:], AF.Gelu,
                                         bias=b1_t[:, kk:kk + 1], scale=1.0)
                    for m in range(KD):
                        nc.tensor.matmul(
                            acc[m][:], lhsT=w2t[:, m * P:(m + 1) * P],
                            rhs=gl[:], start=(kk == 0), stop=(kk == KF - 1))
                for m in range(KD):
                    tmp = ffs.tile([P, 512], F32, name="tmp", tag="tmp")
                    nc.vector.tensor_scalar_add(tmp[:], acc[m][:],
                                                b2_t[:, m:m + 1])
                    nc.vector.tensor_add(hT[m][:, sl], hT[m][:, sl], tmp[:])
        _ln_feature_major(nc, tc, hT, hbf, ones128b, ones1, ln2s_t, ln2b_t, epsc)


def _crf_combine(nc, out_ap, a_ap, b_ap, spool, npart, npair, stabilize=True):
    """out = A 'logmatmul' B over pairs: out[i,j] = lse_k(A[i,k]+B[k,j]).

    a_ap/b_ap: [npart, npair, 81] views ([i,k] / [k,j] row-major).
    out_ap: [npart, npair, 81] view ([i,j] row-major).
    stabilize=False skips the max-subtraction: valid while |entries| stay
    well inside fp32 exp range (in-chunk levels: |x| < ~50).
    """
    s = spool.tile([P, 4, 729], F32, name="cS", tag="cS")
    sv4 = s[:npart, :npair, :].rearrange("p q (x k) -> p q x k", k=T)
    sv3 = s[:npart, :npair, :]
    # ISA TensorTensor allows at most 3 free dims -> emit one add per pair
    for q in range(npair):
        avq = a_ap[:, q].rearrange("p (i k) -> p i k", i=T)
        avq = avq.unsqueeze(2).broadcast_to([npart, T, T, T])    # p i j k
        bvq = b_ap[:, q].rearrange("p (k j) -> p k j", k=T)
        bvq = bvq.unsqueeze(1).broadcast_to([npart, T, T, T])    # p i k j
        bvq = bvq.transpose([0, 1, 3, 2])                        # p i j k
        svq = s[:npart, q, :].rearrange("p (i j k) -> p i j k", i=T, j=T)
        nc.vector.tensor_tensor(out=svq, in0=avq, in1=bvq, op=ALU.add)
    sm = spool.tile([P, 4, 81], F32, name="cR", tag="cR")
    sm3 = sm[:npart, :npair, :]
    if stabilize:
        mx = spool.tile([P, 4, 81], F32, name="cM", tag="cM")
        mx3 = mx[:npart, :npair, :]
        nc.vector.reduce_max(out=mx3, in_=sv4, axis=AX.X)
        mxv = mx3.unsqueeze(3).broadcast_to([npart, npair, 81, T])
        nc.vector.tensor_tensor(out=sv4, in0=sv4, in1=mxv, op=ALU.subtract)
        nc.scalar.activation(sv3, sv3, AF.Exp)
        nc.vector.reduce_sum(out=sm3, in_=sv4, axis=AX.X)
        nc.scalar.activation(sm3, sm3, AF.Ln)
        nc.vector.tensor_tensor(out=out_ap, in0=sm3, in1=mx3, op=ALU.add)
    else:
        nc.scalar.activation(sv3, sv3, AF.Exp)
        nc.vector.reduce_sum(out=sm3, in_=sv4, axis=AX.X)
        nc.scalar.activation(out_ap, sm3, AF.Ln)


def _crf(nc, tc, t):
    """Log-domain associative scan. Partitions 0..63 = example0 chunks,
    64..127 = example1 chunks; each chunk = G=8 consecutive scan steps."""
    with tc.tile_pool(name="crf", bufs=1) as crf, \
         tc.tile_pool(name="crfs", bufs=1) as crfs:
        transB = crf.tile([P, 81], F32, name="transB", tag="transB")
        nc.sync.dma_start(transB[:], t["transB"][:])
        ilogB = crf.tile([P, 81], F32, name="ilogB", tag="ilogB")
        nc.sync.dma_start(ilogB[:], t["ilogB"][:])
        maskB = crf.tile([P, G], F32, name="maskB", tag="maskB")
        nc.sync.dma_start(maskB[:], t["maskB"][:])

        # emS[C] = em rows C*8+2 .. C*8+10 flattened (72 floats per chunk);
        # chunks land on partitions in bit-reversed order via permC so the
        # cross-chunk tree always combines contiguous partition halves.
        shifted = AP(t["em"].tensor, 2 * T, [[G * T, P], [1, G * T]])
        nc.sync.dma_start(t["emS"][:], shifted)
        permt = crf.tile([P, 1], I32, name="permt", tag="permt")
        nc.sync.dma_start(permt[:], t["permC"][:])
        e2 = crf.tile([P, G * T], F32, name="e2", tag="e2")
        nc.gpsimd.indirect_dma_start(
            out=e2[:], out_offset=None, in_=t["emS"][:],
            in_offset=bass.IndirectOffsetOnAxis(ap=permt[:, :1], axis=0),
        )

        # M[c, g, i, j] = ilog + mask*(trans + e - ilog)
        m0 = crf.tile([P, G, 81], F32, name="m0", tag="m0")
        mv = m0[:].rearrange("p g (i j) -> p g i j", i=T)
        e2v = e2[:].rearrange("p (g j) -> p g j", g=G)
        e2v = e2v.unsqueeze(2).broadcast_to([P, G, T, T])
        trv = transB[:].rearrange("p (i j) -> p i j", i=T)
        trv = trv.unsqueeze(1).broadcast_to([P, G, T, T])
        nc.vector.tensor_tensor(out=mv, in0=trv, in1=e2v, op=ALU.add)
        ilv = ilogB[:].rearrange("p (i j) -> p i j", i=T)
        ilv = ilv.unsqueeze(1).broadcast_to([P, G, T, T])
        nc.vector.tensor_tensor(out=mv, in0=mv, in1=ilv, op=ALU.subtract)
        mkv = maskB[:].unsqueeze(2).unsqueeze(3).broadcast_to([P, G, T, T])
        nc.vector.tensor_tensor(out=mv, in0=mv, in1=mkv, op=ALU.mult)
        nc.vector.tensor_tensor(out=mv, in0=mv, in1=ilv, op=ALU.add)

        # in-chunk sequential combines: 8 -> 4 -> 2 -> 1 matrices per chunk
        cur = m0
        width = G
        lvl = 0
        while width > 1:
            width //= 2
            nxt = crf.tile([P, width, 81], F32, name=f"ml{lvl}", tag=f"ml{lvl}")
            pairs = cur[:].rearrange("p (q x) -> p q x", x=81) \
                if cur[:].ndim == 2 else cur[:].rearrange("p a x -> p a x")
            av = pairs[:, 0:2 * width:2, :]
            bv = pairs[:, 1:2 * width:2, :]
            _crf_combine(nc, nxt[:], av, bv, crfs, P, width, stabilize=False)
            cur = nxt
            lvl += 1

        # cross-chunk binary tree over 128 bit-reversed chunk slots; stop at 2
        # (slot 0 = example 0 product, slot 1 = example 1 product)
        nact = P
        cur_ap = cur[:].rearrange("p a x -> p (a x)")   # [128, 81]
        while nact > 2:
            half = nact // 2
            bT = crf.tile([P, 81], F32, name=f"tb{nact}", tag=f"tb{nact}")
            nc.sync.dma_start(bT[:half, :], cur_ap[half:nact])
            nxt = crf.tile([P, 81], F32, name=f"tn{nact}", tag=f"tn{nact}")
            _crf_combine(nc,
                         nxt[:half].unsqueeze(1),
                         cur_ap[:half].unsqueeze(1),
                         bT[:half].unsqueeze(1),
                         crfs, half, 1)
            cur_ap = nxt[:]
            nact = half

        # alpha0 = start + em[:, row 1]; alphaF = alpha0 'logvecmat' Ptot
        a0 = crf.tile([BL, T], F32, name="a0", tag="a0")
        src0 = AP(t["em"].tensor, T, [[S * T, BL], [1, T]])
        nc.sync.dma_start(a0[:], src0)
        st2 = crf.tile([BL, T], F32, name="st2", tag="st2")
        nc.sync.dma_start(st2[:], t["start2"][:])
        nc.vector.tensor_add(a0[:], a0[:], st2[:])

        s0 = crf.tile([BL, T, T], F32, name="s0", tag="s0")   # [b, j, k]
        a0v = a0[:].unsqueeze(1).broadcast_to([BL, T, T])          # k inner
        pv = cur_ap[:BL].rearrange("p (k j) -> p k j", k=T)
        pv = pv.transpose([0, 2, 1])                               # [b, j, k]
        nc.vector.tensor_tensor(out=s0[:], in0=a0v, in1=pv, op=ALU.add)
        mx0 = crf.tile([BL, T], F32, name="mx0", tag="mx0")
        nc.vector.reduce_max(out=mx0[:], in_=s0[:], axis=AX.X)
        mx0v = mx0[:].unsqueeze(2).broadcast_to([BL, T, T])
        nc.vector.tensor_tensor(out=s0[:], in0=s0[:], in1=mx0v,
                                op=ALU.subtract)
        nc.scalar.activation(s0[:], s0[:], AF.Exp)
        sm0 = crf.tile([BL, T], F32, name="sm0", tag="sm0")
        nc.vector.reduce_sum(out=sm0[:], in_=s0[:], axis=AX.X)
        nc.scalar.activation(sm0[:], sm0[:], AF.Ln)
        af = crf.tile([BL, T], F32, name="af", tag="af")
        nc.vector.tensor_add(af[:], sm0[:], mx0[:])
        en2 = crf.tile([BL, T], F32, name="en2", tag="en2")
        nc.sync.dma_start(en2[:], t["end2"][:])
        nc.vector.tensor_add(af[:], af[:], en2[:])
        # logZ = lse over j
        mx1 = crf.tile([BL, 1], F32, name="mx1", tag="mx1")
        nc.vector.reduce_max(out=mx1[:], in_=af[:], axis=AX.X)
        nc.vector.tensor_scalar(out=af[:], in0=af[:], scalar1=mx1[:, :1],
                                scalar2=None, op0=ALU.subtract)
        nc.scalar.activation(af[:], af[:], AF.Exp)
        sm1 = crf.tile([BL, 1], F32, name="sm1", tag="sm1")
        nc.vector.reduce_sum(out=sm1[:], in_=af[:], axis=AX.X)
        nc.scalar.activation(sm1[:], sm1[:], AF.Ln)
        lz = crf.tile([BL, 1], F32, name="lz", tag="lz")
        nc.vector.tensor_add(lz[:], sm1[:], mx1[:])
        nc.sync.dma_start(t["logz"][:], lz[:])


# ----------------------------------------------------------------------------
# host side
# ----------------------------------------------------------------------------

_NC_CACHE = None
last_exec_time_ns = None


def _get_nc():
    global _NC_CACHE
    if _NC_CACHE is None:
        _NC_CACHE = build_program()
    return _NC_CACHE


def _prep_inputs(inputs):
    """Build the 8 per-core input maps (numpy only)."""
    bf = ml_dtypes.bfloat16
    f32 = np.float32
    x = np.asarray(inputs["x"]).astype(np.int64)
    y = np.asarray(inputs["y"]).astype(np.int64)
    g = {k: np.asarray(v).astype(f32) for k, v in inputs.items()
         if k not in ("x", "y")}

    shared = {}
    shared["wemb"] = g["word_emb"]
    shared["pos"] = g["pos_emb"]
    shared["lnesB"] = np.broadcast_to(g["ln_e_s"], (P, D)).copy()
    shared["lnebB"] = np.broadcast_to(g["ln_e_b"], (P, D)).copy()
    shared["wqkv"] = g["Wqkv"].astype(bf)
    shared["wo"] = g["Wo"].astype(bf)
    shared["w1"] = g["W1"].astype(bf)
    shared["w2"] = g["W2"].astype(bf)
    shared["wtag"] = g["W_tag"].astype(bf)
    shared["bqkvT"] = g["bqkv"].reshape(L, 18, P).transpose(0, 2, 1).copy()
    shared["bvB"] = np.broadcast_to(
        g["bqkv"][:, None, 2 * D:], (L, P, D)).copy()
    shared["boT"] = g["bo"].reshape(L, KD, P).transpose(0, 2, 1).copy()
    shared["b1T"] = g["b1"].reshape(L, KF, P).transpose(0, 2, 1).copy()
    shared["b2T"] = g["b2"].reshape(L, KD, P).transpose(0, 2, 1).copy()
    shared["ln1sT"] = g["ln1_s"].reshape(L, KD, P).transpose(0, 2, 1).copy()
    shared["ln1bT"] = g["ln1_b"].reshape(L, KD, P).transpose(0, 2, 1).copy()
    shared["ln2sT"] = g["ln2_s"].reshape(L, KD, P).transpose(0, 2, 1).copy()
    shared["ln2bT"] = g["ln2_b"].reshape(L, KD, P).transpose(0, 2, 1).copy()
    shared["btag"] = g["b_tag"].reshape(T, 1).copy()
    trans = g["crf_trans"]
    shared["transB"] = np.broadcast_to(trans.reshape(1, 81), (P, 81)).copy()
    ilog = np.full((T, T), NEG, f32)
    np.fill_diagonal(ilog, 0.0)
    shared["ilogB"] = np.broadcast_to(ilog.reshape(1, 81), (P, 81)).copy()
    shared["start2"] = np.broadcast_to(g["crf_start"], (BL, T)).copy()
    shared["permC"] = _BITREV7.reshape(P, 1).astype(np.int32)
    shared["end2"] = np.broadcast_to(g["crf_end"], (BL, T)).copy()

    in_maps = []
    num_consts = []
    for c in range(NCORES):
        xs = x[c * BL:(c + 1) * BL]           # [BL, S]
        ys = y[c * BL:(c + 1) * BL]
        m = {}
        m.update(shared)
        m["tok"] = np.ascontiguousarray(
            xs.reshape(NTOK, 1).astype(np.int32))

        tags = ys[:, 1:]                       # [BL, 511]
        mask = (tags > 0)
        mf = mask.astype(f32)
        # scan-step mask: step s uses m[:, s+1], s = 0..509; pad to 512
        mrow = np.zeros((BL, CCH * G), f32)
        mrow[:, :NSTEP] = mf[:, 1:]
        m["maskB"] = np.ascontiguousarray(
            mrow.reshape(BL * CCH, G)[_BITREV7])
        # gold-path emission selection weights
        sel = np.zeros((BL, S, T), f32)
        bi = np.arange(BL)[:, None]
        tpos = np.arange(S - 1)[None, :]
        w = np.concatenate([np.ones((BL, 1), f32), mf[:, 1:]], axis=1)
        sel[bi, tpos + 1, tags] = w
        m["selT"] = np.ascontiguousarray(sel.reshape(NTOK, T).T)
        in_maps.append(m)

        # host part of the numerator (depends only on tags + crf params)
        tr = trans[tags[:, :-1], tags[:, 1:]]
        num_c = g["crf_start"][tags[:, 0]].sum()
        num_c += (tr * mf[:, 1:]).sum()
        last = mask.sum(axis=1).astype(np.int64) - 1
        num_c += g["crf_end"][tags[np.arange(BL), last]].sum()
        num_consts.append(float(num_c))
    return in_maps, num_consts


def kernel(**inputs):
    global last_exec_time_ns
    import os
    nc = _get_nc()
    in_maps, num_consts = _prep_inputs(inputs)
    trace = bool(int(os.environ.get("KERNEL_TRACE", "0")))
    if trace:
        # artifact upload needs bucket creds we may not have; keep it local
        import concourse.bass_utils as _BU
        _BU.upload_artifacts = lambda tmpdir: tmpdir
        try:
            res = run_bass_kernel_spmd(
                nc, in_maps, core_ids=list(range(NCORES)), trace=True)
        except Exception as e:
            print(f"trace run failed ({e!r}); retrying untraced")
            res = run_bass_kernel_spmd(
                nc, in_maps, core_ids=list(range(NCORES)), trace=False)
    else:
        res = run_bass_kernel_spmd(
            nc, in_maps, core_ids=list(range(NCORES)), trace=False)
    last_exec_time_ns = res.exec_time_ns
    loss = 0.0
    for c in range(NCORES):
        r = res.results[c]
        num = num_consts[c] + float(r["numdot"].sum())
        logz = float(r["logz"].sum())
        loss += logz - num
    return np.float32(loss)

